# revision 40
# baseline (speedup 1.0000x reference)
"""BitNet attention TRN2 kernel: 8-core SPMD (2 batch groups x 4 head groups).

Per core cid = 4*g + j (g = batch index, j = head-group index):
  - ternary-quantized QKV projections for heads [4j, 4j+4) of batch g,
    single-pass fp32r matmuls (ternary weights exact in fp32r; activations
    keep ~13 mantissa bits -- enough given the softmax flip-noise budget),
  - attention: fp32r scores, exact row max, exp on ScalarE, probs scaled to
    fp16 and transposed via xbar DMA (no PE/PSUM involvement),
  - attn@v and output projection in fp16 (11-bit mantissa),
  - attn.mean over the core's 4 heads accumulated in fp16 on DVE,
    ReduceScattered per 512-row block over the 4-core batch group,
  - attended values AllGathered (fp16) per 512-column block; the output
    projection for that block overlaps the next block's attention.
BitNet per-tensor scales: each core reduces |w| over a distinct 256-row slab
of each weight; one tiny 8-core AllReduce yields the full-tensor means.
"""

import os

import numpy as np

os.environ.setdefault("NEURON_RT_RESET_CORES", "1")

B, S, D, H = 2, 2048, 2048, 16
HD = D // H            # 128 head dim
HG = H // 4            # 4 heads per core
OS = HG * HD           # 512-wide output slice per core
P = 128
NCORES = 8
NDT = D // P           # 16 contraction tiles
C_SCALE = np.float32(1.0 / np.sqrt(HD))
THRESH = np.float32(2.0 / 3.0)

_CACHE = {}


def _build(use_mask: bool, single: bool = False, phases: int = 3):
    import concourse.mybir as mybir
    import concourse.tile as tile
    from concourse import bacc
    from concourse.masks import make_identity

    F32 = mybir.dt.float32
    F32R = mybir.dt.float32r
    F16 = mybir.dt.float16
    BF16 = mybir.dt.bfloat16
    I32 = mybir.dt.int32
    AX = mybir.AxisListType
    ALU = mybir.AluOpType
    ACTF = mybir.ActivationFunctionType

    nc = bacc.Bacc("TRN2", target_bir_lowering=False, debug=False,
                   num_devices=1 if single else NCORES)

    def cc(kind, op, groups, ins, outs):
        if not single:
            nc.gpsimd.collective_compute(kind, op, replica_groups=groups,
                                         ins=ins, outs=outs)
            return
        # timing-only single-core substitute: local DMA of this core's part
        src_ap, dst_ap = ins[0], outs[0]
        if kind == "AllGather":
            nc.gpsimd.dma_start(out=dst_ap[0:src_ap.shape[0]], in_=src_ap)
        elif kind == "ReduceScatter":
            nc.gpsimd.dma_start(out=dst_ap, in_=src_ap[0:dst_ap.shape[0]])
        else:
            nc.gpsimd.dma_start(out=dst_ap, in_=src_ap)

    # ---- I/O ----
    xq_d = nc.dram_tensor("xq", [S, D], F32, kind="ExternalInput")
    xk_d = nc.dram_tensor("xk", [S, D], F32, kind="ExternalInput")
    xv_d = nc.dram_tensor("xv", [S, D], F32, kind="ExternalInput")
    wslab_d = nc.dram_tensor("w_slab", [4, 256, D], F32, kind="ExternalInput")
    w_in = {
        "q": nc.dram_tensor("wq_s", [OS, D], F32, kind="ExternalInput"),
        "k": nc.dram_tensor("wk_s", [OS, D], F32, kind="ExternalInput"),
        "v": nc.dram_tensor("wv_s", [OS, D], F32, kind="ExternalInput"),
        "o": nc.dram_tensor("wo_s", [OS, D], F32, kind="ExternalInput"),
    }
    bq_d = nc.dram_tensor("bq_s", [P, HG], F32, kind="ExternalInput")
    bk_d = nc.dram_tensor("bk_s", [P, HG], F32, kind="ExternalInput")
    bv_d = nc.dram_tensor("bv_s", [P, HG], F32, kind="ExternalInput")
    bo_d = nc.dram_tensor("bo_s", [1, OS], F32, kind="ExternalInput")
    if use_mask:
        mask_d = nc.dram_tensor("mask_g", [1, S], I32, kind="ExternalInput")
    out_d = nc.dram_tensor("out_slice", [S, OS], F32, kind="ExternalOutput")
    # 4 qb blocks x 128 rows each; host maps block qb to row qb*512+128*j
    mean_d = nc.dram_tensor("mean_slice", [OS, S], BF16, kind="ExternalOutput")

    groups8 = [[0, 1, 2, 3, 4, 5, 6, 7]]
    groups4 = [[0, 1, 2, 3], [4, 5, 6, 7]]
    WIDX = {"q": 0, "k": 1, "v": 2, "o": 3}

    with tile.TileContext(nc) as tc:
        with tc.tile_pool(name="dram", bufs=1, space="DRAM") as dram, \
             tc.tile_pool(name="const", bufs=1) as const:

            # internal DRAM staging
            cc_in = dram.tile([4], F32)
            cc_out = dram.tile([4], F32)
            attT_part = [dram.tile([OS, 512], BF16, name=f"attT_part{i}")
                         for i in range(4)]
            attT_full = [dram.tile([S, 512], BF16, name=f"attT_full{i}")
                         for i in range(4)]
            mean_part = [dram.tile([512, S], BF16, name=f"mean_part{i}")
                         for i in range(4)]
            mean_rs = [dram.tile([P, S], BF16, name=f"mean_rs{i}")
                       for i in range(4)]

            # constants
            ident_f = const.tile([P, P], F32)
            make_identity(nc, ident_f[:])
            ident_r = const.tile([P, P], F32R)
            nc.vector.tensor_copy(out=ident_r[:], in_=ident_f[:])
            ones128 = const.tile([P, 1], F32)
            nc.vector.memset(ones128[:], 1.0)
            ones1h = const.tile([1, P], BF16)
            nc.vector.memset(ones1h[:], 1.0)

            bias_sb = {}
            for nm, d in (("q", bq_d), ("k", bk_d), ("v", bv_d)):
                t = const.tile([P, HG], F32, name=f"bias_{nm}")
                nc.scalar.dma_start(out=t[:], in_=d.ap()[:])
                bias_sb[nm] = t
            bo_row = const.tile([1, OS], F32)
            nc.scalar.dma_start(out=bo_row[:], in_=bo_d.ap()[:])
            bo_hi = const.tile([1, OS], BF16)
            nc.scalar.copy(out=bo_hi[:], in_=bo_row[:])
            bo_lo = const.tile([1, OS], BF16)
            nc.vector.tensor_tensor(out=bo_lo[:], in0=bo_row[:],
                                    in1=bo_hi[:], op=ALU.subtract)

            # ---------- Phase W: |w| slab sums -> AllReduce -> scales ----------
            acc4 = const.tile([P, 4], F32)
            with tc.tile_pool(name="slab", bufs=2) as slabp, \
                 tc.tile_pool(name="w0psum", bufs=1, space="PSUM") as w0p:
                for wi in range(4):
                    sl = slabp.tile([P, 2, D], F32, tag="slab")
                    nc.scalar.dma_start(
                        out=sl[:],
                        in_=wslab_d.ap()[wi].rearrange("(ss p) d -> p ss d",
                                                       p=P))
                    dummy = slabp.tile([P, 2, D], F32, tag="dummy")
                    nc.scalar.activation(dummy[:], sl[:], ACTF.Abs,
                                         accum_out=acc4[:, wi:wi + 1])
                ps4 = w0p.tile([4, 1], F32, tag="ps4")
                nc.tensor.matmul(ps4[:], acc4[:], ones128[:], start=True,
                                 stop=True)
                sums_sb = const.tile([4, 1], F32)
                nc.scalar.copy(out=sums_sb[:], in_=ps4[:])
            nc.sync.dma_start(out=cc_in[:], in_=sums_sb[:])
            cc("AllReduce", ALU.add, groups8, [cc_in[:]], [cc_out[:]])
            rsum = const.tile([1, 4], F32)
            nc.sync.dma_start(out=rsum[:], in_=cc_out[:])

            scale4 = const.tile([1, 4], F32)
            nc.vector.tensor_scalar(out=scale4[:], in0=rsum[:],
                                    scalar1=float(np.float32(1.0 / (D * D))),
                                    scalar2=1e-5, op0=ALU.mult, op1=ALU.max)
            nc.vector.tensor_scalar(out=scale4[:], in0=scale4[:],
                                    scalar1=1000.0, scalar2=None, op0=ALU.min)
            thr4 = const.tile([1, 4], F32)
            nc.vector.tensor_scalar(out=thr4[:], in0=scale4[:],
                                    scalar1=float(THRESH), scalar2=None,
                                    op0=ALU.mult)
            nthr4 = const.tile([1, 4], F32)
            nc.vector.tensor_scalar(out=nthr4[:], in0=thr4[:], scalar1=-1.0,
                                    scalar2=None, op0=ALU.mult)
            scale_c4 = const.tile([1, 4], F32)
            nc.vector.tensor_scalar(out=scale_c4[:], in0=scale4[:],
                                    scalar1=float(C_SCALE), scalar2=None,
                                    op0=ALU.mult)

            def bcast(src_ap, name):
                t = const.tile([P, 1], F32, name=name)
                nc.gpsimd.partition_broadcast(t[:], src_ap)
                return t

            thr_bc = [bcast(thr4[:, wi:wi + 1], f"thr{wi}")
                      for wi in range(4)]
            nthr_bc = [bcast(nthr4[:, wi:wi + 1], f"nthr{wi}")
                       for wi in range(4)]
            sc_bc = [bcast(scale4[:, wi:wi + 1], f"sc{wi}")
                     for wi in range(4)]
            scq_bc = bcast(scale_c4[:, 0:1], "scqc")

            # ---------- ternarize one weight -> fp16 wT via xbar DMA ----------
            # tern = ((w >= -t) - 1) + (w > t)  in {-1, 0, 1}
            W2 = D // 2

            def tern_compute(nm, wT_tile, scratch, upcast, ch, hf):
                wi = WIDX[nm]
                dsl = slice(hf * W2, (hf + 1) * W2)
                wnat = scratch.tile([P, W2], F32, tag="wnat")
                nc.scalar.dma_start(
                    out=wnat[:],
                    in_=w_in[nm].ap()[ch * P:(ch + 1) * P, dsl])
                tmp = scratch.tile([P, W2], BF16, tag="terntmp")
                nc.vector.tensor_scalar(out=tmp[:], in0=wnat[:],
                                        scalar1=nthr_bc[wi][:],
                                        scalar2=-1.0, op0=ALU.is_ge,
                                        op1=ALU.add)
                gt = scratch.tile([P, W2], BF16, tag="terngt")
                nc.gpsimd.tensor_scalar(out=gt[:], in0=wnat[:],
                                        scalar1=thr_bc[wi][:],
                                        scalar2=None, op0=ALU.is_gt)
                tern = scratch.tile([P, W2], BF16, tag="tern")
                nc.vector.tensor_tensor(out=tern[:], in0=tmp[:],
                                        in1=gt[:], op=ALU.add)
                dst = wT_tile[:, hf * 8:(hf + 1) * 8,
                              ch * P:(ch + 1) * P]
                return tern, dst, upcast, scratch

            def tern_flush(st):
                tern, dst, upcast, scratch = st
                if not upcast:
                    nc.sync.dma_start_transpose(out=dst, in_=tern[:])
                else:
                    stg = scratch.tile([P, 8, P], BF16, tag="wstg")
                    nc.sync.dma_start_transpose(out=stg[:], in_=tern[:])
                    nc.vector.tensor_copy(out=dst, in_=stg[:])

            def ternarize16(nm, wT_tile, scratch, upcast):
                for ch in range(4):
                    for hf in range(2):
                        tern_flush(tern_compute(nm, wT_tile, scratch,
                                                upcast, ch, hf))

            with tc.tile_pool(name="kv", bufs=1) as kvp, \
                 tc.tile_pool(name="wo16", bufs=1) as wop:
                qT = kvp.tile([P, HG, S], F16)       # [d', h, s]
                kT = kvp.tile([P, HG, S], F16)
                v_sb = kvp.tile([P, 16, OS], BF16)    # [s_p, st, o]
                woT = wop.tile([P, NDT, OS], BF16)

                # ---------- Phase X: projections ----------
                with tc.tile_pool(name="wtr", bufs=2) as wtrp, \
                     tc.tile_pool(name="xnat", bufs=2) as xnatp, \
                     tc.tile_pool(name="xt", bufs=1) as xtp, \
                     tc.tile_pool(name="wscratch", bufs=2) as wscr, \
                     tc.tile_pool(name="pxt", bufs=4, space="PSUM") as pxt, \
                     tc.tile_pool(name="pmm", bufs=4, space="PSUM") as pmm:

                    wT_next = {}
                    tern_pend = []
                    seq = (("q", xq_d), ("k", xk_d), ("v", xv_d))
                    for wi_, (nm, x_d) in enumerate(seq):
                        if nm in wT_next:
                            wT = wT_next.pop(nm)
                        else:
                            wT = wtrp.tile([P, NDT, OS], F32R, tag="wT",
                                           name=f"wT_{nm}")
                            ternarize16(nm, wT, wscr, upcast=True)
                        # next weight's ternarize interleaves this one's
                        # x-loop (one chunk per (sb, ss) pair)
                        if wi_ + 1 < len(seq):
                            nxt = seq[wi_ + 1][0]
                            wT_next[nxt] = wtrp.tile([P, NDT, OS], F32R,
                                                     tag="wT",
                                                     name=f"wT_{nxt}")
                            nxt_t = wT_next[nxt]
                            nxt_up = True
                        elif phases >= 3:
                            nxt, nxt_t, nxt_up = "o", woT, False
                        else:
                            nxt = None
                        for sb in range(4):
                            xT = xtp.tile([P, NDT, 512], F32R, tag="xT")
                            for ss in range(4):
                                if tern_pend:
                                    tern_flush(tern_pend.pop(0))
                                if nxt is not None and (sb * 4 + ss) % 2 == 0:
                                    ci = (sb * 4 + ss) // 2
                                    tern_pend.append(tern_compute(
                                        nxt, nxt_t, wscr, nxt_up,
                                        ci // 2, ci % 2))
                                r0 = sb * 512 + ss * P
                                xc = xnatp.tile([P, D], F32, tag="xc")
                                nc.sync.dma_start(
                                    out=xc[:], in_=x_d.ap()[r0:r0 + P, :])
                                for dtg in range(4):
                                    pt = pxt.tile([P, 512], F32, tag="xtp")
                                    for di in range(4):
                                        dt_i = dtg * 4 + di
                                        nc.tensor.transpose(
                                            pt[:, di * P:(di + 1) * P],
                                            xc[:, dt_i * P:(dt_i + 1) * P],
                                            ident_f[:])
                                    dst = xT[:, dtg * 4:dtg * 4 + 4,
                                             ss * P:(ss + 1) * P]
                                    psrc = pt[:].rearrange(
                                        "p (di s) -> p di s", di=4)
                                    eng = (nc.scalar, nc.vector,
                                           nc.scalar, nc.vector)[dtg]
                                    if eng is nc.scalar:
                                        nc.scalar.copy(out=dst, in_=psrc)
                                    else:
                                        eng.tensor_copy(out=dst, in_=psrc)
                            if nm == "v":
                                for st_i in range(4):
                                    pp = pmm.tile([P, OS], F32, tag="pp")
                                    stl = slice(st_i * P, (st_i + 1) * P)
                                    for dt_i in range(NDT):
                                        nc.tensor.matmul(
                                            pp[:], xT[:, dt_i, stl],
                                            wT[:, dt_i, :],
                                            start=(dt_i == 0),
                                            stop=(dt_i == NDT - 1))
                                    nc.scalar.activation(
                                        v_sb[:, sb * 4 + st_i, :], pp[:],
                                        ACTF.Copy, scale=sc_bc[2][:])
                            else:
                                th = qT if nm == "q" else kT
                                for ot in range(HG):
                                    pp = pmm.tile([P, 512], F32, tag="pp")
                                    for dt_i in range(NDT):
                                        nc.tensor.matmul(
                                            pp[:],
                                            wT[:, dt_i, ot * P:(ot + 1) * P],
                                            xT[:, dt_i, :],
                                            start=(dt_i == 0),
                                            stop=(dt_i == NDT - 1))
                                    bias = (bias_sb["q"] if nm == "q"
                                            else bias_sb["k"])
                                    scl = scq_bc if nm == "q" else sc_bc[1]
                                    nc.scalar.activation(
                                        th[:, ot, sb * 512:(sb + 1) * 512],
                                        pp[:], ACTF.Identity,
                                        bias=bias[:, ot:ot + 1],
                                        scale=scl[:])

                # ---------- Phase A+O: attention + fused output proj ----------
                if phases >= 2:
                    from contextlib import ExitStack
                    with ExitStack() as stk:
                        pool = tc.tile_pool
                        accp = stk.enter_context(pool(name="accp", bufs=2))
                        maskp = stk.enter_context(pool(name="maskp", bufs=1))
                        p16up = stk.enter_context(pool(name="p16u", bufs=2))
                        p16p = stk.enter_context(pool(name="p16", bufs=2))
                        pTp = stk.enter_context(pool(name="pT", bufs=2))
                        atttp = stk.enter_context(pool(name="attts", bufs=2))
                        attcp = stk.enter_context(pool(name="attc", bufs=2))
                        outsp = stk.enter_context(pool(name="outs", bufs=2))
                        smaxp = stk.enter_context(pool(name="smax", bufs=4))
                        mxp = stk.enter_context(pool(name="mx", bufs=2))
                        scp = stk.enter_context(
                            pool(name="scp", bufs=3, space="PSUM"))
                        avp = stk.enter_context(
                            pool(name="avp", bufs=1, space="PSUM"))
                        pop = stk.enter_context(
                            pool(name="pop", bufs=1, space="PSUM"))

                        if use_mask:
                            mbias = maskp.tile([P, S], F32, tag="mbias")
                            nc.gpsimd.dma_start(out=mbias[0:1, :],
                                                in_=mask_d.ap()[:])
                            nc.vector.tensor_scalar(
                                out=mbias[0:1, :], in0=mbias[0:1, :],
                                scalar1=-1.0, scalar2=1e9,
                                op0=ALU.add, op1=ALU.mult)
                            nc.gpsimd.partition_broadcast(mbias[:],
                                                          mbias[0:1, :])

                        def emit_attnv(h, probsT, attT_sb):
                            pav = avp.tile([P, 512], F32, tag="av")
                            for kt in range(16):
                                nc.tensor.matmul(
                                    pav[:],
                                    v_sb[:, kt, h * P:(h + 1) * P],
                                    probsT[:, kt, :],
                                    start=(kt == 0), stop=(kt == 15))
                            nc.scalar.activation(
                                attT_sb[:, h, :], pav[:], ACTF.Identity,
                                bias=bias_sb["v"][:, h:h + 1],
                                scale=16.0)

                        def emit_qb_finish(qb, acc16, attT_sb):
                            nc.sync.dma_start(
                                out=attT_part[qb][:].rearrange(
                                    "(h p) q -> p h q", p=P),
                                in_=attT_sb[:])
                            cc("AllGather", ALU.bypass, groups4,
                               [attT_part[qb][:]], [attT_full[qb][:]])
                            for qt in range(4):
                                nc.sync.dma_start(
                                    out=mean_part[qb]
                                    [qt * P:(qt + 1) * P, :],
                                    in_=acc16[:, qt, :])
                            cc("ReduceScatter", ALU.add, groups4,
                               [mean_part[qb][:]], [mean_rs[qb][:]])
                            nc.sync.dma_start(
                                out=mean_d.ap()[qb * P:(qb + 1) * P, :],
                                in_=mean_rs[qb][:])

                        def emit_outproj(qb):
                            attc = attcp.tile([P, NDT, 512], BF16,
                                              tag="attc")
                            nc.sync.dma_start(
                                out=attc[:],
                                in_=attT_full[qb][:]
                                .rearrange("(dt p) s -> p dt s", p=P))
                            for st_i in range(4):
                                stl = slice(st_i * P, (st_i + 1) * P)
                                po = pop.tile([P, OS], F32, tag="po")
                                for dt_i in range(NDT):
                                    nc.tensor.matmul(
                                        po[:], attc[:, dt_i, stl],
                                        woT[:, dt_i, :],
                                        start=(dt_i == 0), stop=False)
                                nc.tensor.matmul(po[:], ones1h[:],
                                                 bo_hi[:],
                                                 start=False, stop=False)
                                nc.tensor.matmul(po[:], ones1h[:],
                                                 bo_lo[:],
                                                 start=False, stop=True)
                                osb = outsp.tile([P, OS], F32, tag="osb")
                                nc.scalar.activation(osb[:], po[:],
                                                     ACTF.Copy,
                                                     scale=sc_bc[3][:])
                                r0 = (qb * 4 + st_i) * P
                                nc.sync.dma_start(
                                    out=out_d.ap()[r0:r0 + P, :],
                                    in_=osb[:])

                        prev = None  # (qb, h, probsT, acc16, attT_sb)
                        for qb in range(4):
                            acc16 = accp.tile([P, 4, S], BF16, tag="acc")
                            attT_sb = atttp.tile([P, HG, 512], BF16,
                                                 tag="attT")
                            for h in range(HG):
                                probsT = pTp.tile([P, NDT, 512], BF16,
                                                  tag="pT")
                                for qt in range(4):
                                    psc = [scp.tile([P, 1024], F32, tag="sc",
                                                    name=f"sc{hf}")
                                           for hf in range(2)]
                                    q0c = qb * 512
                                    qcol = slice(q0c + qt * P,
                                                 q0c + (qt + 1) * P)
                                    den2 = smaxp.tile([P, 2], F32, tag="den2")
                                    p16u = p16up.tile([P, S], BF16, tag="p16u")
                                    for kb in range(4):
                                        nc.tensor.matmul(
                                            psc[kb // 2]
                                            [:, (kb % 2) * 512:
                                             (kb % 2) * 512 + 512],
                                            qT[:, h, qcol],
                                            kT[:, h, kb * 512:(kb + 1) * 512],
                                            start=True, stop=True)
                                    if use_mask:
                                        for hf in range(2):
                                            nc.vector.tensor_tensor(
                                                out=psc[hf][:],
                                                in0=psc[hf][:],
                                                in1=mbias[:, hf * 1024:
                                                          (hf + 1) * 1024],
                                                op=ALU.add)
                                    # per-tile row max (DVE, PSUM-read),
                                    # negated; combine via min -> -rowmax
                                    nmh = smaxp.tile([P, 2], F32, tag="nmh")
                                    for hf in range(2):
                                        nc.vector.tensor_reduce(
                                            out=nmh[:, hf:hf + 1],
                                            in_=psc[hf][:], axis=AX.X,
                                            op=ALU.max, negate=True)
                                    nmx = smaxp.tile([P, 1], F32, tag="nmx")
                                    nc.vector.tensor_reduce(
                                        out=nmx[:], in_=nmh[:], axis=AX.X,
                                        op=ALU.min)
                                    for hf in range(2):
                                        nc.scalar.activation(
                                            p16u[:, hf * 1024:
                                                 (hf + 1) * 1024],
                                            psc[hf][:], ACTF.Exp,
                                            bias=nmx[:], scale=1.0,
                                            accum_out=den2[:, hf:hf + 1])
                                    den16 = smaxp.tile([P, 1], F32,
                                                       tag="den16")
                                    nc.vector.tensor_reduce(
                                        out=den16[:], in_=den2[:], axis=AX.X,
                                        op=ALU.add)
                                    r16 = smaxp.tile([P, 1], F32, tag="r16")
                                    nc.vector.reciprocal(out=r16[:],
                                                         in_=den16[:])
                                    if h == 0:
                                        def p16ap(sl_):
                                            return acc16[:, qt, sl_]
                                    else:
                                        p16t = p16p.tile([P, S], BF16,
                                                         tag="p16")

                                        def p16ap(sl_):
                                            return p16t[:, sl_]
                                    for hf in range(2):
                                        hfs = slice(hf * 1024,
                                                    (hf + 1) * 1024)
                                        nc.vector.tensor_scalar(
                                            out=p16ap(hfs),
                                            in0=p16u[:, hfs],
                                            scalar1=r16[:], scalar2=0.0625,
                                            op0=ALU.mult, op1=ALU.mult)
                                        nc.sync.dma_start_transpose(
                                            out=probsT[:, hf * 8:hf * 8 + 8,
                                                       qt * P:(qt + 1) * P],
                                            in_=p16ap(hfs))
                                    if h > 0:
                                        nc.vector.tensor_tensor(
                                            out=acc16[:, qt, :],
                                            in0=acc16[:, qt, :],
                                            in1=p16t[:], op=ALU.add)
                                # one step behind: attn@v of the previous
                                # head runs while this head's softmax drains
                                if prev is not None:
                                    pqb, ph, ppT, pacc, patt = prev
                                    emit_attnv(ph, ppT, patt)
                                    if ph == HG - 1:
                                        emit_qb_finish(pqb, pacc, patt)
                                if h == 2 and qb >= 1 and phases >= 3:
                                    emit_outproj(qb - 1)
                                prev = (qb, h, probsT, acc16, attT_sb)
                        pqb, ph, ppT, pacc, patt = prev
                        emit_attnv(ph, ppT, patt)
                        emit_qb_finish(pqb, pacc, patt)
                        if phases >= 3:
                            emit_outproj(3)

    nc.compile()
    return nc


def kernel(**inputs):
    query = np.ascontiguousarray(inputs["query"], dtype=np.float32)
    key = np.ascontiguousarray(inputs["key"], dtype=np.float32)
    value = np.ascontiguousarray(inputs["value"], dtype=np.float32)
    mask = np.asarray(inputs["mask"])
    ws = {n: np.ascontiguousarray(inputs[n], dtype=np.float32)
          for n in ("wq", "wk", "wv", "wo")}
    bs = {n: np.ascontiguousarray(inputs[n], dtype=np.float32)
          for n in ("bq", "bk", "bv", "bo")}

    use_mask = not bool(np.all(mask == 1))
    if use_mask not in _CACHE:
        _CACHE[use_mask] = _build(use_mask)
    nc = _CACHE[use_mask]

    in_maps = []
    for cid in range(NCORES):
        g, j = divmod(cid, 4)
        sl = slice(OS * j, OS * (j + 1))
        m = {
            "xq": query[g], "xk": key[g], "xv": value[g],
            "w_slab": np.stack([ws[n][256 * cid:256 * (cid + 1), :]
                                for n in ("wq", "wk", "wv", "wo")]),
            "wq_s": ws["wq"][sl], "wk_s": ws["wk"][sl],
            "wv_s": ws["wv"][sl], "wo_s": ws["wo"][sl],
            "bq_s": np.ascontiguousarray(bs["bq"][sl].reshape(HG, P).T),
            "bk_s": np.ascontiguousarray(bs["bk"][sl].reshape(HG, P).T),
            "bv_s": np.ascontiguousarray(bs["bv"][sl].reshape(HG, P).T),
            "bo_s": bs["bo"][sl].reshape(1, OS),
        }
        if use_mask:
            m["mask_g"] = np.ascontiguousarray(
                mask[g], dtype=np.int32).reshape(1, S)
        in_maps.append(m)

    global _last_in_maps
    _last_in_maps = in_maps

    from concourse.bass_utils import run_bass_kernel_spmd
    tdir = os.environ.get("BASS_KERNEL_TRACE_DIR")
    tkw = {"trace": True, "tmpdir": tdir} if tdir else {}
    res = run_bass_kernel_spmd(nc, in_maps, core_ids=list(range(NCORES)), **tkw)

    out = np.empty((B, S, D), np.float32)
    attn_mean = np.empty((B, S, S), np.float32)
    for cid in range(NCORES):
        g, j = divmod(cid, 4)
        out[g][:, OS * j:OS * (j + 1)] = res.results[cid]["out_slice"]
        ms = res.results[cid]["mean_slice"].astype(np.float32)
        for qb in range(4):
            r0 = qb * 512 + P * j
            attn_mean[g][r0:r0 + P, :] = ms[qb * P:(qb + 1) * P]
    return out, attn_mean


# revision 41
# speedup vs baseline: 1.2328x; 1.2328x over previous
"""BitNet attention TRN2 kernel: 8-core SPMD (2 batch groups x 4 head groups).

Per core cid = 4*g + j (g = batch index, j = head-group index):
  - ternary-quantized QKV projections for heads [4j, 4j+4) of batch g,
    single-pass fp32r matmuls (ternary weights exact in fp32r; activations
    keep ~13 mantissa bits -- enough given the softmax flip-noise budget),
  - attention: fp32r scores, exact row max, exp on ScalarE, probs scaled to
    fp16 and transposed via xbar DMA (no PE/PSUM involvement),
  - attn@v and output projection in fp16 (11-bit mantissa),
  - attn.mean over the core's 4 heads accumulated in fp16 on DVE,
    ReduceScattered per 512-row block over the 4-core batch group,
  - attended values AllGathered (fp16) per 512-column block; the output
    projection for that block overlaps the next block's attention.
BitNet per-tensor scales: each core reduces |w| over a distinct 256-row slab
of each weight; one tiny 8-core AllReduce yields the full-tensor means.
"""

import os

import numpy as np

os.environ.setdefault("NEURON_RT_RESET_CORES", "1")

B, S, D, H = 2, 2048, 2048, 16
HD = D // H            # 128 head dim
HG = H // 4            # 4 heads per core
OS = HG * HD           # 512-wide output slice per core
P = 128
NCORES = 8
NDT = D // P           # 16 contraction tiles
C_SCALE = np.float32(1.0 / np.sqrt(HD))
THRESH = np.float32(2.0 / 3.0)

_CACHE = {}


def _build(use_mask: bool, single: bool = False, phases: int = 3):
    import concourse.mybir as mybir
    import concourse.tile as tile
    from concourse import bacc
    from concourse.masks import make_identity

    F32 = mybir.dt.float32
    F32R = mybir.dt.float32r
    F16 = mybir.dt.float16
    BF16 = mybir.dt.bfloat16
    I32 = mybir.dt.int32
    AX = mybir.AxisListType
    ALU = mybir.AluOpType
    ACTF = mybir.ActivationFunctionType

    nc = bacc.Bacc("TRN2", target_bir_lowering=False, debug=False,
                   num_devices=1 if single else NCORES)

    def cc(kind, op, groups, ins, outs):
        if not single:
            nc.gpsimd.collective_compute(kind, op, replica_groups=groups,
                                         ins=ins, outs=outs)
            return
        # timing-only single-core substitute: local DMA of this core's part
        src_ap, dst_ap = ins[0], outs[0]
        if kind == "AllGather":
            nc.gpsimd.dma_start(out=dst_ap[0:src_ap.shape[0]], in_=src_ap)
        elif kind == "ReduceScatter":
            nc.gpsimd.dma_start(out=dst_ap, in_=src_ap[0:dst_ap.shape[0]])
        else:
            nc.gpsimd.dma_start(out=dst_ap, in_=src_ap)

    # ---- I/O ----
    xq_d = nc.dram_tensor("xq", [S, D], F32, kind="ExternalInput")
    xk_d = nc.dram_tensor("xk", [S, D], F32, kind="ExternalInput")
    xv_d = nc.dram_tensor("xv", [S, D], F32, kind="ExternalInput")
    wslab_d = nc.dram_tensor("w_slab", [4, 256, D], F32, kind="ExternalInput")
    w_in = {
        "q": nc.dram_tensor("wq_s", [OS, D], F32, kind="ExternalInput"),
        "k": nc.dram_tensor("wk_s", [OS, D], F32, kind="ExternalInput"),
        "v": nc.dram_tensor("wv_s", [OS, D], F32, kind="ExternalInput"),
        "o": nc.dram_tensor("wo_s", [OS, D], F32, kind="ExternalInput"),
    }
    bq_d = nc.dram_tensor("bq_s", [P, HG], F32, kind="ExternalInput")
    bk_d = nc.dram_tensor("bk_s", [P, HG], F32, kind="ExternalInput")
    bv_d = nc.dram_tensor("bv_s", [P, HG], F32, kind="ExternalInput")
    bo_d = nc.dram_tensor("bo_s", [1, OS], F32, kind="ExternalInput")
    if use_mask:
        mask_d = nc.dram_tensor("mask_g", [1, S], I32, kind="ExternalInput")
    out_d = nc.dram_tensor("out_slice", [S, OS], F32, kind="ExternalOutput")
    # per-core partial head-sum; host sums the 4 cores of a batch group
    mean_d = nc.dram_tensor("mean_slice", [S, S], BF16, kind="ExternalOutput")

    groups8 = [[0, 1, 2, 3, 4, 5, 6, 7]]
    groups4 = [[0, 1, 2, 3], [4, 5, 6, 7]]
    WIDX = {"q": 0, "k": 1, "v": 2, "o": 3}

    with tile.TileContext(nc) as tc:
        with tc.tile_pool(name="dram", bufs=1, space="DRAM") as dram, \
             tc.tile_pool(name="const", bufs=1) as const:

            # internal DRAM staging
            cc_in = dram.tile([4], F32)
            cc_out = dram.tile([4], F32)
            attT_part = [dram.tile([OS, 512], BF16, name=f"attT_part{i}")
                         for i in range(4)]
            attT_full = [dram.tile([S, 512], BF16, name=f"attT_full{i}")
                         for i in range(4)]

            # constants
            ident_f = const.tile([P, P], F32)
            make_identity(nc, ident_f[:])
            ident_r = const.tile([P, P], F32R)
            nc.vector.tensor_copy(out=ident_r[:], in_=ident_f[:])
            ones128 = const.tile([P, 1], F32)
            nc.vector.memset(ones128[:], 1.0)
            ones1h = const.tile([1, P], BF16)
            nc.vector.memset(ones1h[:], 1.0)

            bias_sb = {}
            for nm, d in (("q", bq_d), ("k", bk_d), ("v", bv_d)):
                t = const.tile([P, HG], F32, name=f"bias_{nm}")
                nc.scalar.dma_start(out=t[:], in_=d.ap()[:])
                bias_sb[nm] = t
            bo_row = const.tile([1, OS], F32)
            nc.scalar.dma_start(out=bo_row[:], in_=bo_d.ap()[:])
            bo_hi = const.tile([1, OS], BF16)
            nc.scalar.copy(out=bo_hi[:], in_=bo_row[:])
            bo_lo = const.tile([1, OS], BF16)
            nc.vector.tensor_tensor(out=bo_lo[:], in0=bo_row[:],
                                    in1=bo_hi[:], op=ALU.subtract)

            # ---------- Phase W: |w| slab sums -> AllReduce -> scales ----------
            acc4 = const.tile([P, 4], F32)
            with tc.tile_pool(name="slab", bufs=2) as slabp, \
                 tc.tile_pool(name="w0psum", bufs=1, space="PSUM") as w0p:
                for wi in range(4):
                    sl = slabp.tile([P, 2, D], F32, tag="slab")
                    nc.scalar.dma_start(
                        out=sl[:],
                        in_=wslab_d.ap()[wi].rearrange("(ss p) d -> p ss d",
                                                       p=P))
                    dummy = slabp.tile([P, 2, D], F32, tag="dummy")
                    nc.scalar.activation(dummy[:], sl[:], ACTF.Abs,
                                         accum_out=acc4[:, wi:wi + 1])
                ps4 = w0p.tile([4, 1], F32, tag="ps4")
                nc.tensor.matmul(ps4[:], acc4[:], ones128[:], start=True,
                                 stop=True)
                sums_sb = const.tile([4, 1], F32)
                nc.scalar.copy(out=sums_sb[:], in_=ps4[:])
            nc.sync.dma_start(out=cc_in[:], in_=sums_sb[:])
            cc("AllReduce", ALU.add, groups8, [cc_in[:]], [cc_out[:]])
            rsum = const.tile([1, 4], F32)
            nc.sync.dma_start(out=rsum[:], in_=cc_out[:])

            scale4 = const.tile([1, 4], F32)
            nc.vector.tensor_scalar(out=scale4[:], in0=rsum[:],
                                    scalar1=float(np.float32(1.0 / (D * D))),
                                    scalar2=1e-5, op0=ALU.mult, op1=ALU.max)
            nc.vector.tensor_scalar(out=scale4[:], in0=scale4[:],
                                    scalar1=1000.0, scalar2=None, op0=ALU.min)
            thr4 = const.tile([1, 4], F32)
            nc.vector.tensor_scalar(out=thr4[:], in0=scale4[:],
                                    scalar1=float(THRESH), scalar2=None,
                                    op0=ALU.mult)
            nthr4 = const.tile([1, 4], F32)
            nc.vector.tensor_scalar(out=nthr4[:], in0=thr4[:], scalar1=-1.0,
                                    scalar2=None, op0=ALU.mult)
            scale_c4 = const.tile([1, 4], F32)
            nc.vector.tensor_scalar(out=scale_c4[:], in0=scale4[:],
                                    scalar1=float(C_SCALE), scalar2=None,
                                    op0=ALU.mult)

            def bcast(src_ap, name):
                t = const.tile([P, 1], F32, name=name)
                nc.gpsimd.partition_broadcast(t[:], src_ap)
                return t

            thr_bc = [bcast(thr4[:, wi:wi + 1], f"thr{wi}")
                      for wi in range(4)]
            nthr_bc = [bcast(nthr4[:, wi:wi + 1], f"nthr{wi}")
                       for wi in range(4)]
            sc_bc = [bcast(scale4[:, wi:wi + 1], f"sc{wi}")
                     for wi in range(4)]
            scq_bc = bcast(scale_c4[:, 0:1], "scqc")

            # ---------- ternarize one weight -> fp16 wT via xbar DMA ----------
            # tern = ((w >= -t) - 1) + (w > t)  in {-1, 0, 1}
            W2 = D // 2

            def tern_compute(nm, wT_tile, scratch, upcast, ch, hf):
                wi = WIDX[nm]
                dsl = slice(hf * W2, (hf + 1) * W2)
                wnat = scratch.tile([P, W2], F32, tag="wnat")
                nc.scalar.dma_start(
                    out=wnat[:],
                    in_=w_in[nm].ap()[ch * P:(ch + 1) * P, dsl])
                tmp = scratch.tile([P, W2], BF16, tag="terntmp")
                nc.vector.tensor_scalar(out=tmp[:], in0=wnat[:],
                                        scalar1=nthr_bc[wi][:],
                                        scalar2=-1.0, op0=ALU.is_ge,
                                        op1=ALU.add)
                gt = scratch.tile([P, W2], BF16, tag="terngt")
                nc.gpsimd.tensor_scalar(out=gt[:], in0=wnat[:],
                                        scalar1=thr_bc[wi][:],
                                        scalar2=None, op0=ALU.is_gt)
                tern = scratch.tile([P, W2], BF16, tag="tern")
                nc.vector.tensor_tensor(out=tern[:], in0=tmp[:],
                                        in1=gt[:], op=ALU.add)
                dst = wT_tile[:, hf * 8:(hf + 1) * 8,
                              ch * P:(ch + 1) * P]
                return tern, dst, upcast, scratch

            def tern_flush(st):
                tern, dst, upcast, scratch = st
                if not upcast:
                    nc.sync.dma_start_transpose(out=dst, in_=tern[:])
                else:
                    stg = scratch.tile([P, 8, P], BF16, tag="wstg")
                    nc.sync.dma_start_transpose(out=stg[:], in_=tern[:])
                    nc.vector.tensor_copy(out=dst, in_=stg[:])

            def ternarize16(nm, wT_tile, scratch, upcast):
                for ch in range(4):
                    for hf in range(2):
                        tern_flush(tern_compute(nm, wT_tile, scratch,
                                                upcast, ch, hf))

            with tc.tile_pool(name="kv", bufs=1) as kvp, \
                 tc.tile_pool(name="wo16", bufs=1) as wop:
                qT = kvp.tile([P, HG, S], F16)       # [d', h, s]
                kT = kvp.tile([P, HG, S], F16)
                v_sb = kvp.tile([P, 16, OS], BF16)    # [s_p, st, o]
                woT = wop.tile([P, NDT, OS], BF16)

                # ---------- Phase X: projections ----------
                with tc.tile_pool(name="wtr", bufs=2) as wtrp, \
                     tc.tile_pool(name="xnat", bufs=2) as xnatp, \
                     tc.tile_pool(name="xt", bufs=1) as xtp, \
                     tc.tile_pool(name="wscratch", bufs=2) as wscr, \
                     tc.tile_pool(name="pxt", bufs=4, space="PSUM") as pxt, \
                     tc.tile_pool(name="pmm", bufs=4, space="PSUM") as pmm:

                    wT_next = {}
                    tern_pend = []
                    seq = (("q", xq_d), ("k", xk_d), ("v", xv_d))
                    for wi_, (nm, x_d) in enumerate(seq):
                        if nm in wT_next:
                            wT = wT_next.pop(nm)
                        else:
                            wT = wtrp.tile([P, NDT, OS], F32R, tag="wT",
                                           name=f"wT_{nm}")
                            ternarize16(nm, wT, wscr, upcast=True)
                        # next weight's ternarize interleaves this one's
                        # x-loop (one chunk per (sb, ss) pair)
                        if wi_ + 1 < len(seq):
                            nxt = seq[wi_ + 1][0]
                            wT_next[nxt] = wtrp.tile([P, NDT, OS], F32R,
                                                     tag="wT",
                                                     name=f"wT_{nxt}")
                            nxt_t = wT_next[nxt]
                            nxt_up = True
                        elif phases >= 3:
                            nxt, nxt_t, nxt_up = "o", woT, False
                        else:
                            nxt = None
                        for sb in range(4):
                            xT = xtp.tile([P, NDT, 512], F32R, tag="xT")
                            for ss in range(4):
                                if tern_pend:
                                    tern_flush(tern_pend.pop(0))
                                if nxt is not None and (sb * 4 + ss) % 2 == 0:
                                    ci = (sb * 4 + ss) // 2
                                    tern_pend.append(tern_compute(
                                        nxt, nxt_t, wscr, nxt_up,
                                        ci // 2, ci % 2))
                                r0 = sb * 512 + ss * P
                                xc = xnatp.tile([P, D], F32, tag="xc")
                                nc.sync.dma_start(
                                    out=xc[:], in_=x_d.ap()[r0:r0 + P, :])
                                for dtg in range(4):
                                    pt = pxt.tile([P, 512], F32, tag="xtp")
                                    for di in range(4):
                                        dt_i = dtg * 4 + di
                                        nc.tensor.transpose(
                                            pt[:, di * P:(di + 1) * P],
                                            xc[:, dt_i * P:(dt_i + 1) * P],
                                            ident_f[:])
                                    dst = xT[:, dtg * 4:dtg * 4 + 4,
                                             ss * P:(ss + 1) * P]
                                    psrc = pt[:].rearrange(
                                        "p (di s) -> p di s", di=4)
                                    eng = (nc.scalar, nc.vector,
                                           nc.scalar, nc.vector)[dtg]
                                    if eng is nc.scalar:
                                        nc.scalar.copy(out=dst, in_=psrc)
                                    else:
                                        eng.tensor_copy(out=dst, in_=psrc)
                            if nm == "v":
                                for st_i in range(4):
                                    pp = pmm.tile([P, OS], F32, tag="pp")
                                    stl = slice(st_i * P, (st_i + 1) * P)
                                    for dt_i in range(NDT):
                                        nc.tensor.matmul(
                                            pp[:], xT[:, dt_i, stl],
                                            wT[:, dt_i, :],
                                            start=(dt_i == 0),
                                            stop=(dt_i == NDT - 1))
                                    nc.scalar.activation(
                                        v_sb[:, sb * 4 + st_i, :], pp[:],
                                        ACTF.Copy, scale=sc_bc[2][:])
                            else:
                                th = qT if nm == "q" else kT
                                for ot in range(HG):
                                    pp = pmm.tile([P, 512], F32, tag="pp")
                                    for dt_i in range(NDT):
                                        nc.tensor.matmul(
                                            pp[:],
                                            wT[:, dt_i, ot * P:(ot + 1) * P],
                                            xT[:, dt_i, :],
                                            start=(dt_i == 0),
                                            stop=(dt_i == NDT - 1))
                                    bias = (bias_sb["q"] if nm == "q"
                                            else bias_sb["k"])
                                    scl = scq_bc if nm == "q" else sc_bc[1]
                                    nc.scalar.activation(
                                        th[:, ot, sb * 512:(sb + 1) * 512],
                                        pp[:], ACTF.Identity,
                                        bias=bias[:, ot:ot + 1],
                                        scale=scl[:])

                # ---------- Phase A+O: attention + fused output proj ----------
                if phases >= 2:
                    from contextlib import ExitStack
                    with ExitStack() as stk:
                        pool = tc.tile_pool
                        accp = stk.enter_context(pool(name="accp", bufs=2))
                        maskp = stk.enter_context(pool(name="maskp", bufs=1))
                        p16up = stk.enter_context(pool(name="p16u", bufs=2))
                        p16p = stk.enter_context(pool(name="p16", bufs=2))
                        pTp = stk.enter_context(pool(name="pT", bufs=2))
                        atttp = stk.enter_context(pool(name="attts", bufs=2))
                        attcp = stk.enter_context(pool(name="attc", bufs=2))
                        outsp = stk.enter_context(pool(name="outs", bufs=2))
                        smaxp = stk.enter_context(pool(name="smax", bufs=4))
                        mxp = stk.enter_context(pool(name="mx", bufs=2))
                        scp = stk.enter_context(
                            pool(name="scp", bufs=3, space="PSUM"))
                        avp = stk.enter_context(
                            pool(name="avp", bufs=1, space="PSUM"))
                        pop = stk.enter_context(
                            pool(name="pop", bufs=1, space="PSUM"))

                        if use_mask:
                            mbias = maskp.tile([P, S], F32, tag="mbias")
                            nc.gpsimd.dma_start(out=mbias[0:1, :],
                                                in_=mask_d.ap()[:])
                            nc.vector.tensor_scalar(
                                out=mbias[0:1, :], in0=mbias[0:1, :],
                                scalar1=-1.0, scalar2=1e9,
                                op0=ALU.add, op1=ALU.mult)
                            nc.gpsimd.partition_broadcast(mbias[:],
                                                          mbias[0:1, :])

                        def emit_attnv(h, probsT, attT_sb):
                            pav = avp.tile([P, 512], F32, tag="av")
                            for kt in range(16):
                                nc.tensor.matmul(
                                    pav[:],
                                    v_sb[:, kt, h * P:(h + 1) * P],
                                    probsT[:, kt, :],
                                    start=(kt == 0), stop=(kt == 15))
                            nc.scalar.activation(
                                attT_sb[:, h, :], pav[:], ACTF.Identity,
                                bias=bias_sb["v"][:, h:h + 1],
                                scale=16.0)

                        def emit_qb_finish(qb, acc16, attT_sb):
                            nc.sync.dma_start(
                                out=attT_part[qb][:].rearrange(
                                    "(h p) q -> p h q", p=P),
                                in_=attT_sb[:])
                            cc("AllGather", ALU.bypass, groups4,
                               [attT_part[qb][:]], [attT_full[qb][:]])
                            for qt in range(4):
                                r0 = qb * 512 + qt * P
                                nc.sync.dma_start(
                                    out=mean_d.ap()[r0:r0 + P, :],
                                    in_=acc16[:, qt, :])

                        def emit_outproj(qb):
                            attc = attcp.tile([P, NDT, 512], BF16,
                                              tag="attc")
                            nc.sync.dma_start(
                                out=attc[:],
                                in_=attT_full[qb][:]
                                .rearrange("(dt p) s -> p dt s", p=P))
                            for st_i in range(4):
                                stl = slice(st_i * P, (st_i + 1) * P)
                                po = pop.tile([P, OS], F32, tag="po")
                                for dt_i in range(NDT):
                                    nc.tensor.matmul(
                                        po[:], attc[:, dt_i, stl],
                                        woT[:, dt_i, :],
                                        start=(dt_i == 0), stop=False)
                                nc.tensor.matmul(po[:], ones1h[:],
                                                 bo_hi[:],
                                                 start=False, stop=False)
                                nc.tensor.matmul(po[:], ones1h[:],
                                                 bo_lo[:],
                                                 start=False, stop=True)
                                osb = outsp.tile([P, OS], F32, tag="osb")
                                nc.scalar.activation(osb[:], po[:],
                                                     ACTF.Copy,
                                                     scale=sc_bc[3][:])
                                r0 = (qb * 4 + st_i) * P
                                nc.sync.dma_start(
                                    out=out_d.ap()[r0:r0 + P, :],
                                    in_=osb[:])

                        prev = None  # (qb, h, probsT, acc16, attT_sb)
                        for qb in range(4):
                            acc16 = accp.tile([P, 4, S], BF16, tag="acc")
                            attT_sb = atttp.tile([P, HG, 512], BF16,
                                                 tag="attT")
                            for h in range(HG):
                                probsT = pTp.tile([P, NDT, 512], BF16,
                                                  tag="pT")
                                for qt in range(4):
                                    psc = [scp.tile([P, 1024], F32, tag="sc",
                                                    name=f"sc{hf}")
                                           for hf in range(2)]
                                    q0c = qb * 512
                                    qcol = slice(q0c + qt * P,
                                                 q0c + (qt + 1) * P)
                                    den2 = smaxp.tile([P, 2], F32, tag="den2")
                                    p16u = p16up.tile([P, S], BF16, tag="p16u")
                                    for kb in range(4):
                                        nc.tensor.matmul(
                                            psc[kb // 2]
                                            [:, (kb % 2) * 512:
                                             (kb % 2) * 512 + 512],
                                            qT[:, h, qcol],
                                            kT[:, h, kb * 512:(kb + 1) * 512],
                                            start=True, stop=True)
                                    if use_mask:
                                        for hf in range(2):
                                            nc.vector.tensor_tensor(
                                                out=psc[hf][:],
                                                in0=psc[hf][:],
                                                in1=mbias[:, hf * 1024:
                                                          (hf + 1) * 1024],
                                                op=ALU.add)
                                    # per-tile row max (DVE, PSUM-read),
                                    # negated; combine via min -> -rowmax
                                    nmh = smaxp.tile([P, 2], F32, tag="nmh")
                                    for hf in range(2):
                                        nc.vector.tensor_reduce(
                                            out=nmh[:, hf:hf + 1],
                                            in_=psc[hf][:], axis=AX.X,
                                            op=ALU.max, negate=True)
                                    nmx = smaxp.tile([P, 1], F32, tag="nmx")
                                    nc.vector.tensor_reduce(
                                        out=nmx[:], in_=nmh[:], axis=AX.X,
                                        op=ALU.min)
                                    for hf in range(2):
                                        nc.scalar.activation(
                                            p16u[:, hf * 1024:
                                                 (hf + 1) * 1024],
                                            psc[hf][:], ACTF.Exp,
                                            bias=nmx[:], scale=1.0,
                                            accum_out=den2[:, hf:hf + 1])
                                    den16 = smaxp.tile([P, 1], F32,
                                                       tag="den16")
                                    nc.vector.tensor_reduce(
                                        out=den16[:], in_=den2[:], axis=AX.X,
                                        op=ALU.add)
                                    r16 = smaxp.tile([P, 1], F32, tag="r16")
                                    nc.vector.reciprocal(out=r16[:],
                                                         in_=den16[:])
                                    if h == 0:
                                        def p16ap(sl_):
                                            return acc16[:, qt, sl_]
                                    else:
                                        p16t = p16p.tile([P, S], BF16,
                                                         tag="p16")

                                        def p16ap(sl_):
                                            return p16t[:, sl_]
                                    for hf in range(2):
                                        hfs = slice(hf * 1024,
                                                    (hf + 1) * 1024)
                                        nc.vector.tensor_scalar(
                                            out=p16ap(hfs),
                                            in0=p16u[:, hfs],
                                            scalar1=r16[:], scalar2=0.0625,
                                            op0=ALU.mult, op1=ALU.mult)
                                        nc.sync.dma_start_transpose(
                                            out=probsT[:, hf * 8:hf * 8 + 8,
                                                       qt * P:(qt + 1) * P],
                                            in_=p16ap(hfs))
                                    if h > 0:
                                        nc.vector.tensor_tensor(
                                            out=acc16[:, qt, :],
                                            in0=acc16[:, qt, :],
                                            in1=p16t[:], op=ALU.add)
                                # one step behind: attn@v of the previous
                                # head runs while this head's softmax drains
                                if prev is not None:
                                    pqb, ph, ppT, pacc, patt = prev
                                    emit_attnv(ph, ppT, patt)
                                    if ph == HG - 1:
                                        emit_qb_finish(pqb, pacc, patt)
                                if h == 2 and qb >= 1 and phases >= 3:
                                    emit_outproj(qb - 1)
                                prev = (qb, h, probsT, acc16, attT_sb)
                        pqb, ph, ppT, pacc, patt = prev
                        emit_attnv(ph, ppT, patt)
                        emit_qb_finish(pqb, pacc, patt)
                        if phases >= 3:
                            emit_outproj(3)

    nc.compile()
    return nc


def kernel(**inputs):
    query = np.ascontiguousarray(inputs["query"], dtype=np.float32)
    key = np.ascontiguousarray(inputs["key"], dtype=np.float32)
    value = np.ascontiguousarray(inputs["value"], dtype=np.float32)
    mask = np.asarray(inputs["mask"])
    ws = {n: np.ascontiguousarray(inputs[n], dtype=np.float32)
          for n in ("wq", "wk", "wv", "wo")}
    bs = {n: np.ascontiguousarray(inputs[n], dtype=np.float32)
          for n in ("bq", "bk", "bv", "bo")}

    use_mask = not bool(np.all(mask == 1))
    if use_mask not in _CACHE:
        _CACHE[use_mask] = _build(use_mask)
    nc = _CACHE[use_mask]

    in_maps = []
    for cid in range(NCORES):
        g, j = divmod(cid, 4)
        sl = slice(OS * j, OS * (j + 1))
        m = {
            "xq": query[g], "xk": key[g], "xv": value[g],
            "w_slab": np.stack([ws[n][256 * cid:256 * (cid + 1), :]
                                for n in ("wq", "wk", "wv", "wo")]),
            "wq_s": ws["wq"][sl], "wk_s": ws["wk"][sl],
            "wv_s": ws["wv"][sl], "wo_s": ws["wo"][sl],
            "bq_s": np.ascontiguousarray(bs["bq"][sl].reshape(HG, P).T),
            "bk_s": np.ascontiguousarray(bs["bk"][sl].reshape(HG, P).T),
            "bv_s": np.ascontiguousarray(bs["bv"][sl].reshape(HG, P).T),
            "bo_s": bs["bo"][sl].reshape(1, OS),
        }
        if use_mask:
            m["mask_g"] = np.ascontiguousarray(
                mask[g], dtype=np.int32).reshape(1, S)
        in_maps.append(m)

    global _last_in_maps
    _last_in_maps = in_maps

    from concourse.bass_utils import run_bass_kernel_spmd
    tdir = os.environ.get("BASS_KERNEL_TRACE_DIR")
    tkw = {"trace": True, "tmpdir": tdir} if tdir else {}
    res = run_bass_kernel_spmd(nc, in_maps, core_ids=list(range(NCORES)), **tkw)

    out = np.empty((B, S, D), np.float32)
    attn_mean = np.zeros((B, S, S), np.float32)
    for cid in range(NCORES):
        g, j = divmod(cid, 4)
        out[g][:, OS * j:OS * (j + 1)] = res.results[cid]["out_slice"]
        attn_mean[g] += res.results[cid]["mean_slice"].astype(np.float32)
    return out, attn_mean
